# revision 92
# baseline (speedup 1.0000x reference)
import sys

sys.path.insert(0, "/opt/trn_rl_repo")

import numpy as np
import ml_dtypes

# Phi3SeerAttention, B=1 S=2048 HIDDEN=3072, H=32 q heads, HK=8 kv heads,
# D=96, gate block 64, gate hidden 128. Sharded TP over kv heads: core c
# owns kv head c and q heads 4c..4c+3; o-proj row-sharded, partials summed
# on host (the gather step).
#
# Software-pipelined chunk schedule (chunk = 512 tokens): during chunk j's
# attention, the PE filler stream carries o-proj(j-1) plus the ENTIRE prep
# of chunk j+1 (QKV, gate, RoPE), so the gate's long cross-engine chain
# never stalls the in-order PE queue at chunk boundaries.
#
# The block-sparse gate mask is folded into the score matmul itself:
# k_sb rows 96:128 hold a constant one-hot k-block indicator and q_sb rows
# 96:128 hold per-chunk bias rows (0 = active block, -60 = masked), so
# exp(scores) comes out pre-masked with no per-tile mask multiply.
H, HK, D, BLK, GH = 32, 8, 96, 64, 128
S, HIDDEN = 2048, 3072
G = H // HK          # 4 q heads per kv head (per core)
NB = S // BLK        # 32 gate blocks
KT = HIDDEN // 128   # 24 contraction tiles
NS = S // 512        # 4 sequence chunks of 512
NT = S // 128        # 16 t-tiles of 128
NE = HIDDEN // 512   # 6 output column chunks
QK = G * D + D       # 480 packed q+k output dims (q = rows 0..383, k = 384..479)
NCORES = 8
THR = 0.03

_prog = None


def _build(debug=False):
    from concourse import bass, mybir, bacc
    from concourse.bass import AP
    import concourse.tile as tile
    from contextlib import ExitStack

    dt = mybir.dt
    BF, F32 = dt.bfloat16, dt.float32
    AF = mybir.ActivationFunctionType
    OP = mybir.AluOpType
    AX = mybir.AxisListType.X

    nc = bacc.Bacc()
    F8 = dt.float8e4
    PM = mybir.MatmulPerfMode.DoubleRow
    xth_d = nc.dram_tensor("xth", [HIDDEN, S], F8, kind="ExternalInput")
    xtl_d = nc.dram_tensor("xtl", [HIDDEN, S], F8, kind="ExternalInput")
    wqkh_d = nc.dram_tensor("wqkh", [HIDDEN, QK], F8, kind="ExternalInput")
    wqkl_d = nc.dram_tensor("wqkl", [HIDDEN, QK], F8, kind="ExternalInput")
    wvh_d = nc.dram_tensor("wvh", [HIDDEN, D], F8, kind="ExternalInput")
    wvl_d = nc.dram_tensor("wvl", [HIDDEN, D], F8, kind="ExternalInput")
    owp_d = nc.dram_tensor("owp", [128, 6 * HIDDEN], F8, kind="ExternalInput")
    cosq_d = nc.dram_tensor("cosq", [D, S], BF, kind="ExternalInput")
    sinq_d = nc.dram_tensor("sinq", [D, S], BF, kind="ExternalInput")
    cosk_d = nc.dram_tensor("cosk", [D, S], BF, kind="ExternalInput")
    sink_d = nc.dram_tensor("sink", [D, S], BF, kind="ExternalInput")
    gwqp_d = nc.dram_tensor("gwqp", [128, 3 * GH], F32, kind="ExternalInput")
    gwk_d = nc.dram_tensor("gwk", [2 * D, GH], F32, kind="ExternalInput")
    eye8_d = nc.dram_tensor("eye8", [8, 8], F32, kind="ExternalInput")
    eyer_d = nc.dram_tensor("eyer", [8, NS * NB], F32, kind="ExternalInput")
    kext_d = nc.dram_tensor("kext", [NB, S], BF, kind="ExternalInput")
    bcm_d = nc.dram_tensor("bcm", [8, NS * NB], F32, kind="ExternalInput")
    cmask_d = nc.dram_tensor("cmask", [128, 512], BF, kind="ExternalInput")
    out_d = nc.dram_tensor("out_p", [S, HIDDEN], BF, kind="ExternalOutput")

    # de-interleave map: packed q row r = 96*h + d lives in tile r//128,
    # partition r%128.  pieces[h] = [(tile, psrc0, dsrc0, len), ...]
    qpieces = {
        0: [(0, 0, 0, 96)],
        1: [(0, 96, 0, 32), (1, 0, 32, 64)],
        2: [(1, 64, 0, 64), (2, 0, 64, 32)],
        3: [(2, 32, 0, 96)],
    }

    with tile.TileContext(nc) as tc:
        with ExitStack() as ctx:
            perm = ctx.enter_context(tc.tile_pool(name="perm", bufs=1))
            # weights / tables (hi/lo fp8 residual pairs; weights carry a
            # x64 scale that is divided back out downstream)
            wqkh_sb = perm.tile([128, KT, QK], F8)
            wqkl_sb = perm.tile([128, KT, QK], F8)
            wvh_sb = perm.tile([128, KT, D], F8)
            wvl_sb = perm.tile([128, KT, D], F8)
            # o-proj in fp8 3-term: owp slots 0-2 = hi, 3-5 = lo (x64);
            # attnp slots 0-2 = hi, 3-5 = lo, 6 = zeros (DoubleRow filler)
            owp_sb = perm.tile([128, 6, HIDDEN], F8)
            gwqp_sb = perm.tile([128, 3, GH], F32)
            gwk_sb = perm.tile([D, 2, GH], F32)
            shuf_sb = perm.tile([D, G + 1, 512], BF)
            cosq_sb = perm.tile([D, S], BF)
            sinq_sb = perm.tile([D, S], BF)
            cosk_sb = perm.tile([D, S], BF)
            sink_sb = perm.tile([D, S], BF)
            eye8_sb = perm.tile([8, 8], F32)
            eyer_sb = perm.tile([8, NS, NB], F32)
            bcm_sb = perm.tile([8, NS, NB], F32)
            cmask_sb = perm.tile([128, 512], BF)
            ones_sb = perm.tile([1, 128], BF)
            # activations (q_sb/k_sb carry 32 extra mask-bias rows)
            q_sb = perm.tile([D + NB, G, S], BF)
            k_sb = perm.tile([D + NB, S], BF)
            v_sb = perm.tile([128, NT, D + 1], BF)
            qkp_sb = perm.tile([128, 3, 512], BF)   # packed q of current chunk
            qp_sb = perm.tile([128, 3, NB], F32)    # packed q block-sums
            km_sb = perm.tile([D, NB], F32)
            kx_sb = perm.tile([D, NB], F32)
            qgT_sb = perm.tile([GH, NB], F32)
            kgT_sb = perm.tile([GH, NB], F32)
            attnp_sb = perm.tile([128, 7, S], F8)
            xth_sb = perm.tile([128, 2, KT, 512], F8)  # double-buffered x^T
            xtl_sb = perm.tile([128, 2, KT, 512], F8)

            # priority order: the first QKV matmuls need xt chunk0 + wqk;
            # interleave 3-kt batches of both so PE can start ~2.5us in.
            groups = [2, 2, 2, 3, 3, 3, 3, 3, 3]
            # hi/lo streams interleaved per kt-group: x pair on sync,
            # weight pair on gpsimd; the prologue's pair-level emission
            # chases both.  Tables/smalls get the scalar queue to
            # themselves so RoPE never waits.
            k0 = 0
            for kb in groups:
                ks = slice(k0 * 128, (k0 + kb) * 128)
                for xs, xd in ((xth_sb, xth_d), (xtl_sb, xtl_d)):
                    nc.sync.dma_start(
                        xs[:, 0, k0 : k0 + kb, :],
                        xd[ks, 0:512].rearrange("(k p) c -> p k c", p=128),
                    )
                for ws, wd in ((wqkh_sb, wqkh_d), (wqkl_sb, wqkl_d)):
                    nc.gpsimd.dma_start(
                        ws[:, k0 : k0 + kb, :],
                        wd[ks, :].rearrange("(k p) c -> p k c", p=128),
                    )
                k0 += kb
            nc.scalar.dma_start(cosq_sb[:], cosq_d[:])
            nc.scalar.dma_start(sinq_sb[:], sinq_d[:])
            nc.scalar.dma_start(cosk_sb[:], cosk_d[:])
            nc.scalar.dma_start(sink_sb[:], sink_d[:])
            for t in range(3):
                nc.scalar.dma_start(
                    gwqp_sb[:, t, :], gwqp_d[:, t * GH : (t + 1) * GH]
                )
            nc.scalar.dma_start(gwk_sb[:, 0, :], gwk_d[0:D, :])
            nc.scalar.dma_start(gwk_sb[:, 1, :], gwk_d[D : 2 * D, :])
            nc.scalar.dma_start(eye8_sb[:], eye8_d[:])
            nc.scalar.dma_start(eyer_sb[:], eyer_d[:])
            nc.scalar.dma_start(k_sb[D : D + NB, :], kext_d[:])
            nc.scalar.dma_start(bcm_sb[:], bcm_d[:])
            nc.scalar.dma_start(cmask_sb[:], cmask_d[:])
            nc.scalar.dma_start(
                wvh_sb[:], wvh_d[:].rearrange("(k p) c -> p k c", p=128)
            )
            nc.scalar.dma_start(
                wvl_sb[:], wvl_d[:].rearrange("(k p) c -> p k c", p=128)
            )
            nc.vector.memset(ones_sb[:], 1.0)
            nc.vector.memset(v_sb[:, :, D : D + 1], 1.0)
            nc.vector.memset(q_sb[D : D + NB, :, :], 0.0)
            nc.vector.memset(attnp_sb[:, 6, :], 0.0)
            # warm the ACT function tables while the engines idle during
            # the initial loads (first real use would stall ~1.3us)
            warm_sb = perm.tile([1, 4], F32)
            nc.vector.memset(warm_sb[:], 1.0)
            nc.scalar.activation(warm_sb[:, 0:1], warm_sb[:, 1:2], AF.Exp)

            # PSUM: 2x 1-bank rotating (qkv/o-proj/gate/norm), 2x 2-bank
            # score pairs, 2x 1-bank PV accumulators = 8 banks
            psR = ctx.enter_context(
                tc.tile_pool(name="psR", bufs=2, space="PSUM")
            )
            psPV = ctx.enter_context(
                tc.tile_pool(name="psPV", bufs=2, space="PSUM")
            )

            osb = ctx.enter_context(tc.tile_pool(name="osb", bufs=2))
            rope_sb = ctx.enter_context(tc.tile_pool(name="rope", bufs=2))
            pfull = ctx.enter_context(tc.tile_pool(name="pfull", bufs=6))
            gsb = ctx.enter_context(tc.tile_pool(name="gsb", bufs=1))
            nsb = ctx.enter_context(tc.tile_pool(name="nsb", bufs=2))

            # ---------------- prep: QKV / gate / RoPE of one chunk --------
            def v_tile(ti):
                def th():
                    ps = psR.tile([128, 512], F32, tag="ps", name=f"v_ps{ti}")
                    pv = ps[:, :D]
                    vxb = (ti // 4) % 2
                    csl = slice((ti % 4) * 128, (ti % 4 + 1) * 128)
                    terms = [
                        (xth_sb, wvh_sb),
                        (xth_sb, wvl_sb),
                        (xtl_sb, wvh_sb),
                    ]
                    n = len(terms) * (KT // 2)
                    i = 0
                    for xs, ws in terms:
                        for p2 in range(KT // 2):
                            nc.tensor.matmul(
                                pv,
                                xs[:, vxb, 2 * p2 : 2 * p2 + 2, csl],
                                ws[:, 2 * p2 : 2 * p2 + 2, :],
                                start=(i == 0),
                                stop=(i == n - 1),
                                perf_mode=PM,
                                skip_group_check=True,
                            )
                            i += 1
                    nc.scalar.activation(
                        v_sb[:, ti, :D], pv, AF.Copy, scale=1.0 / 64.0
                    )
                return th

            def qkv_thunks(jj):
                # 4 thunks: 3 packed q tiles + 1 k tile; reduces +
                # psum->SBUF copies inline so the psum slot frees fast.
                jbs = slice(jj * 8, (jj + 1) * 8)
                jsl = slice(jj * 512, (jj + 1) * 512)
                jxb = jj % 2

                def mk(t):
                    def th():
                        ps = psR.tile(
                            [128, 512], F32, tag="ps", name=f"qkv{jj}_{t}"
                        )
                        csl = (
                            slice(t * 128, (t + 1) * 128)
                            if t < 3
                            else slice(384, 480)
                        )
                        po = ps[:, :512] if t < 3 else ps[:D, :]
                        terms = [
                            (wqkh_sb, xth_sb),
                            (wqkh_sb, xtl_sb),
                            (wqkl_sb, xth_sb),
                        ]
                        n = len(terms) * (KT // 2)
                        i = 0
                        for ws, xs in terms:
                            for p2 in range(KT // 2):
                                nc.tensor.matmul(
                                    po,
                                    ws[:, 2 * p2 : 2 * p2 + 2, csl],
                                    xs[:, jxb, 2 * p2 : 2 * p2 + 2, :],
                                    start=(i == 0),
                                    stop=(i == n - 1),
                                    perf_mode=PM,
                                    skip_group_check=True,
                                )
                                i += 1
                        if t < 3:
                            pr = ps[:].rearrange("p (b w) -> p b w", w=BLK)
                            nc.vector.tensor_reduce(
                                qp_sb[:, t, jbs], pr, axis=AX, op=OP.add
                            )
                            if t in (0, 2):
                                nc.scalar.copy(qkp_sb[:, t, :], ps[:])
                            else:
                                nc.vector.tensor_copy(qkp_sb[:, t, :], ps[:])
                        else:
                            prk = ps[:D, :].rearrange("p (b w) -> p b w", w=BLK)
                            nc.vector.tensor_reduce(
                                km_sb[:, jbs], prk, axis=AX, op=OP.add
                            )
                            nc.vector.tensor_reduce(
                                kx_sb[:, jbs], prk, axis=AX, op=OP.max
                            )
                            nc.vector.tensor_copy(k_sb[0:D, jsl], ps[:D, :])
                    return th

                return [mk(t) for t in range(4)]

            def gate_thunks(jj):
                # 4 thunks; each PE bit only fires after its ACT/DVE inputs
                # have had several fill-slots worth of time to land.
                jbs = slice(jj * 8, (jj + 1) * 8)
                jsl = slice(jj * 512, (jj + 1) * 512)
                w = 8 * (jj + 1)
                st = {}

                def ga():
                    qg_ps = psR.tile([GH, 8], F32, tag="ps", name=f"qg{jj}")
                    for t in range(3):
                        nc.tensor.matmul(
                            qg_ps,
                            gwqp_sb[:, t, :],
                            qp_sb[:, t, jbs],
                            start=(t == 0),
                            stop=(t == 2),
                            skip_group_check=True,
                        )
                    nc.scalar.mul(
                        qgT_sb[:, jbs],
                        qg_ps[:],
                        (1.0 / (G * BLK)) * GH**-0.5 / 64.0,
                    )

                def gb():
                    kg_ps = psR.tile([GH, 8], F32, tag="ps", name=f"kg{jj}")
                    nc.tensor.matmul(
                        kg_ps, gwk_sb[:, 0, :], km_sb[:, jbs],
                        start=True, stop=False, skip_group_check=True,
                    )
                    nc.tensor.matmul(
                        kg_ps, gwk_sb[:, 1, :], kx_sb[:, jbs],
                        start=False, stop=True, skip_group_check=True,
                    )
                    nc.scalar.activation(
                        kgT_sb[:, jbs], kg_ps[:], AF.Copy, scale=1.0 / 64.0
                    )

                def gc():
                    lg_ps = psR.tile([8, NB], F32, tag="ps", name=f"lg{jj}")
                    nc.tensor.matmul(
                        lg_ps[:, :w], qgT_sb[:, jbs], kgT_sb[:, :w],
                        start=True, stop=True, skip_group_check=True,
                    )
                    lm = gsb.tile([8, NB], F32)
                    nc.vector.tensor_add(lm[:, :w], lg_ps[:, :w], bcm_sb[:, jj, :w])
                    ge = gsb.tile([8, NB], F32)
                    gsum = gsb.tile([8, 1], F32)
                    nc.scalar.activation(
                        ge[:, :w], lm[:, :w], AF.Exp, accum_out=gsum[:]
                    )
                    grc = gsb.tile([8, 1], F32)
                    nc.vector.reciprocal(grc[:], gsum[:])
                    prob = gsb.tile([8, NB], F32)
                    nc.scalar.activation(prob[:, :w], ge[:, :w], AF.Copy, scale=grc[:])
                    m01 = gsb.tile([8, NB], F32)
                    nc.vector.tensor_scalar(
                        m01[:, :w], prob[:, :w], THR, None, op0=OP.is_ge
                    )
                    nc.vector.tensor_tensor(
                        m01[:, :w], m01[:, :w], eyer_sb[:, jj, :w], op=OP.max
                    )
                    st["m01"] = m01

                def gd():
                    m01 = st["m01"]
                    m01t_ps = psR.tile([NB, 8], F32, tag="ps", name=f"m01t{jj}")
                    nc.tensor.matmul(
                        m01t_ps[:w, :], m01[:, :w], eye8_sb[:],
                        start=True, stop=True, skip_group_check=True,
                    )
                    m01tb = gsb.tile([NB, 8], BF)
                    nc.vector.tensor_scalar(
                        m01tb[:w, :], m01t_ps[:w, :], 60.0, 60.0,
                        op0=OP.mult, op1=OP.subtract,
                    )
                    bsrc = m01tb[0:w, 0:8]
                    bap = AP(
                        tensor=bsrc.tensor,
                        offset=bsrc.offset,
                        ap=[list(bsrc.ap[0]), list(bsrc.ap[1]), [0, BLK]],
                    )
                    for hh, eng in enumerate(
                        [nc.vector, nc.scalar, nc.gpsimd, nc.gpsimd]
                    ):
                        if eng is nc.scalar:
                            eng.activation(
                                q_sb[D : D + w, hh, jsl], bap, AF.Copy
                            )
                        else:
                            eng.tensor_copy(q_sb[D : D + w, hh, jsl], bap)

                return [ga, gb, gc, gd]

            def rope_thunks(jj):
                jsl = slice(jj * 512, (jj + 1) * 512)
                h2 = D // 2
                # k first: every score tile of the chunk contracts with
                # k_sb, so its RoPE gates the whole attention phase
                order = [G] + list(range(G))

                def deint():
                    # de-interleave packed q -> per-head q_sb.  Partition
                    # shifts need DMA (on the Pool queue: 25ns triggers);
                    # head 0's piece is shift-free so DVE copies it.
                    nc.vector.tensor_copy(
                        q_sb[0:D, 0, jsl], qkp_sb[0:D, 0, :]
                    )
                    for hh in range(1, G):
                        for (t, ps0, pd0, ln) in qpieces[hh]:
                            nc.gpsimd.dma_start(
                                q_sb[pd0 : pd0 + ln, hh, jsl],
                                qkp_sb[ps0 : ps0 + ln, t, :],
                            )
                    # rotate-half staging: k first, then all q heads in
                    # two batched DMAs per half
                    nc.gpsimd.dma_start(
                        shuf_sb[0:h2, G, :], k_sb[h2:D, jsl]
                    )
                    nc.gpsimd.dma_start(
                        shuf_sb[h2:D, G, :], k_sb[0:h2, jsl]
                    )
                    nc.gpsimd.dma_start(
                        shuf_sb[0:h2, 0:G, :], q_sb[h2:D, 0:G, jsl]
                    )
                    nc.gpsimd.dma_start(
                        shuf_sb[h2:D, 0:G, :], q_sb[0:h2, 0:G, jsl]
                    )

                def rope_of(hh):
                    src = q_sb[0:D, hh, jsl] if hh < G else k_sb[0:D, jsl]
                    cs = cosq_sb if hh < G else cosk_sb
                    sn = sinq_sb if hh < G else sink_sb
                    t1 = rope_sb.tile([D, 512], BF)
                    nc.vector.tensor_mul(t1[:], src, cs[:, jsl])
                    t2 = rope_sb.tile([D, 512], BF)
                    nc.vector.tensor_mul(t2[:], shuf_sb[:, hh, :], sn[:, jsl])
                    nc.vector.tensor_add(src, t1[:], t2[:])

                def rope1():
                    # k and head 0 first: they gate the chunk's first scores
                    rope_of(G)
                    rope_of(0)

                def rope2():
                    for hh in (1, 2, 3):
                        rope_of(hh)

                return [deint, rope1, rope2]

            def prep_thunks(jj):
                qt = qkv_thunks(jj)
                ga, gb, gc, gd = gate_thunks(jj)
                de, ro1, ro2 = rope_thunks(jj)
                vt = [v_tile(ti) for ti in range(4 * jj, 4 * jj + 4)]
                # order: q tiles, gate A after q reduces, k tile, gate B,
                # v tiles spacing out the gate chain, then bias + rope
                return (
                    qt[0:3]
                    + [ga]
                    + [qt[3]]
                    + [gb, vt[0], vt[1], gc, vt[2], vt[3], gd, de, ro1, ro2]
                )

            def prologue_qkv():
                # chunk 0 runs before anything else, so its QKV is emitted
                # term-by-term ACROSS tiles: the hi*hi terms chase the hi
                # DMA streams kt-by-kt while the lo streams land.  Four
                # accumulators stay open (2 "ps" + 2 borrowed pair slots).
                jbs = slice(0, 8)
                jsl = slice(0, 512)
                ts = []
                for t in range(4):
                    if t < 2:
                        pp = psR.tile(
                            [128, 512], F32, tag="score", bufs=4,
                            name=f"pq{t}",
                        )
                        ts.append(pp[:])
                    else:
                        pp = psR.tile([128, 512], F32, tag="ps", name=f"pq{t}")
                        ts.append(pp[:])
                terms = [
                    (wqkh_sb, xth_sb),
                    (wqkh_sb, xtl_sb),
                    (wqkl_sb, xth_sb),
                ]
                def finish(t):
                    ps = ts[t]
                    if t < 3:
                        pr = ps.rearrange("p (b w) -> p b w", w=BLK)
                        nc.vector.tensor_reduce(
                            qp_sb[:, t, jbs], pr, axis=AX, op=OP.add
                        )
                        nc.scalar.copy(qkp_sb[:, t, :], ps)
                    else:
                        prk = ps[:D, :].rearrange("p (b w) -> p b w", w=BLK)
                        nc.vector.tensor_reduce(
                            km_sb[:, jbs], prk, axis=AX, op=OP.add
                        )
                        nc.vector.tensor_reduce(
                            kx_sb[:, jbs], prk, axis=AX, op=OP.max
                        )
                        nc.scalar.copy(k_sb[0:D, jsl], ps[:D, :])

                def mm(t, term, p2, start, stop):
                    ws, xs = terms[term]
                    csl = (
                        slice(t * 128, (t + 1) * 128)
                        if t < 3
                        else slice(384, 480)
                    )
                    po = ts[t] if t < 3 else ts[t][:D, :]
                    nc.tensor.matmul(
                        po,
                        ws[:, 2 * p2 : 2 * p2 + 2, csl],
                        xs[:, 0, 2 * p2 : 2 * p2 + 2, :],
                        start=start,
                        stop=stop,
                        perf_mode=PM,
                        skip_group_check=True,
                    )

                # phase 1 (pairs 0..7): pair-major across tiles, chasing
                # the interleaved hi/lo DMA streams; phase 2: tile-major so
                # tile completions (and their reduces) stagger
                NP1 = 10
                for p2 in range(NP1):
                    for term in range(3):
                        for t in range(4):
                            mm(t, term, p2, term == 0 and p2 == 0, False)
                for t in range(4):
                    for p2 in range(NP1, KT // 2):
                        for term in range(3):
                            mm(
                                t, term, p2, False,
                                term == 2 and p2 == KT // 2 - 1,
                            )
                    finish(t)

            # ---------------- o-proj thunks ------------------------------
            def oproj_thunks(si):
                tsl = slice(si * 128, (si + 1) * 128)
                o_sb = osb.tile([128, NE, 512], BF, name=f"o_sb{si}", tag="o_sb")
                eng = nc.sync if si % 2 == 0 else nc.gpsimd
                eng2 = nc.gpsimd if si % 2 == 0 else nc.sync
                oview = out_d[tsl, :].rearrange("p (e c) -> p e c", c=512)

                def apair(s1, s2):
                    base = attnp_sb[:, s1, tsl]
                    return AP(
                        tensor=base.tensor,
                        offset=base.offset,
                        ap=[list(base.ap[0]), [(s2 - s1) * S, 2],
                            list(base.ap[1])],
                    )

                def mk(ej):
                    def th():
                        esl = slice(ej * 512, (ej + 1) * 512)

                        def wpair(s1, s2):
                            base = owp_sb[:, s1, esl]
                            return AP(
                                tensor=base.tensor,
                                offset=base.offset,
                                ap=[list(base.ap[0]),
                                    [(s2 - s1) * HIDDEN, 2],
                                    list(base.ap[1])],
                            )

                        o_ps = psR.tile(
                            [128, 512], F32, tag="ps", name=f"o_ps{si}_{ej}"
                        )
                        # 3-term fp8 residual o-proj in 5 DoubleRow pairs:
                        # hi*wh (h0w0,h1w1,h2w2), lo*wh, hi*wl; the odd
                        # ninth product pairs with the zero slot 6
                        pairs = [
                            ((0, 1), (0, 1)),
                            ((2, 3), (2, 0)),
                            ((4, 5), (1, 2)),
                            ((0, 1), (3, 4)),
                            ((2, 6), (5, 3)),
                        ]
                        for i, (aa, ww) in enumerate(pairs):
                            nc.tensor.matmul(
                                o_ps,
                                apair(*aa),
                                wpair(*ww),
                                start=(i == 0),
                                stop=(i == len(pairs) - 1),
                                perf_mode=PM,
                                skip_group_check=True,
                            )
                        if (si + ej) % 3 == 0:
                            nc.scalar.activation(
                                o_sb[:, ej, :], o_ps[:], AF.Copy,
                                scale=1.0 / 64.0,
                            )
                        else:
                            nc.vector.tensor_scalar(
                                o_sb[:, ej, :], o_ps[:], 1.0 / 64.0, None,
                                op0=OP.mult,
                            )
                        if ej == 2:
                            eng.dma_start(oview[:, 0:3, :], o_sb[:, 0:3, :])
                        elif ej == NE - 1:
                            eng2.dma_start(oview[:, 3:6, :], o_sb[:, 3:6, :])
                    return th

                return [mk(ej) for ej in range(NE)]

            # ---------------- attention ----------------------------------
            def attention(j, fills, carried):
                sl = slice(j * 512, (j + 1) * 512)
                ntile = 4 * (j + 1)
                nslots = G * ntile
                nf0 = len(fills)
                consumed = [0]
                slot = [0]

                def tick():
                    # consume fills so they are exhausted by ~80% of slots
                    slot[0] += 1
                    target = min(nf0, (nf0 * slot[0]) // max(1, int(nslots * 0.95)))
                    while consumed[0] < target and fills:
                        fills.pop(0)()
                        consumed[0] += 1

                head_state = {}

                def emit_score(hh, ti):
                    c0 = max(ti - 4 * j, 0) * 128
                    s_ps = psR.tile([128, 512], F32, tag="score", bufs=4)
                    p_sb = pfull.tile([128, 512], BF)
                    nc.tensor.matmul(
                        s_ps[:, c0:],
                        k_sb[:, ti * 128 : (ti + 1) * 128],
                        q_sb[:, hh, j * 512 + c0 : (j + 1) * 512],
                        start=True,
                        stop=True,
                        skip_group_check=True,
                    )
                    nc.scalar.activation(p_sb[:, c0:], s_ps[:, c0:], AF.Exp)
                    if ti - 4 * j >= 0:
                        # only the 128-col diagonal sub-block is partially
                        # masked; past it causal is all-ones.  Pool keeps
                        # this off the congested DVE queue.
                        nc.gpsimd.tensor_tensor(
                            p_sb[:, c0 : c0 + 128],
                            p_sb[:, c0 : c0 + 128],
                            cmask_sb[:, 0:128],
                            op=OP.mult,
                        )
                    head_state[hh][1][ti] = (p_sb, c0)

                def emit_pv(hh, ti):
                    pv_ps, p_tiles = head_state[hh]
                    p_sb, c0 = p_tiles[ti]
                    nc.tensor.matmul(
                        pv_ps[:, c0:],
                        v_sb[:, ti, :],
                        p_sb[:, c0:],
                        start=(ti == 0),
                        stop=(ti == ntile - 1),
                        skip_group_check=True,
                    )
                    p_tiles[ti] = None

                def head_open(hh):
                    head_state[hh] = (
                        psPV.tile([D + 1, 512], F32, tag="pv", name=f"pv{hh}"),
                        [None] * ntile,
                    )

                PIPE = 3

                def head_scores(hh):
                    head_open(hh)
                    for ti in range(min(PIPE, ntile)):
                        emit_score(hh, ti)
                    for ti in range(ntile):
                        if ti + PIPE < ntile:
                            emit_score(hh, ti + PIPE)
                        emit_pv(hh, ti)
                        tick()

                def norm_pre(hh):
                    pv_ps = head_state[hh][0]
                    rcb = nsb.tile([1, 512], BF, tag="rcb")
                    with nc.allow_low_precision(
                        reason="1/Z was already cast to bf16 for the PE broadcast"
                    ):
                        nc.vector.reciprocal(rcb[:], pv_ps[D : D + 1, :])
                    head_state[hh] = (pv_ps, rcb)

                def head_norm(hh):
                    pv_ps, rcb = head_state[hh]
                    do_head_norm(hh, j, pv_ps, rcb)

                pend = None
                for hh in range(G):
                    head_scores(hh)
                    norm_pre(hh)
                    if pend is not None:
                        pend()
                    if hh < 3 or j == NS - 1:
                        pend = (lambda h: lambda: head_norm(h))(hh)
                    else:
                        # defer head 3's norm chain into the next chunk's
                        # fill stream (its reciprocal gets the whole
                        # inter-chunk window to land)
                        pend = None
                        pv_ps, rcb = head_state[3]
                        carried["h3norm"] = (
                            lambda p, r, jj: lambda: do_head_norm(3, jj, p, r)
                        )(pv_ps, rcb, j)
                if pend is not None:
                    pend()
                # drain leftover fills
                while fills:
                    fills.pop(0)()

            def do_head_norm(hh, jj, pv_ps, rcb):
                ssl = slice(jj * 512, (jj + 1) * 512)
                rb_ps = psR.tile([D, 512], F32, tag="ps")
                nc.tensor.matmul(
                    rb_ps, ones_sb[:, :D], rcb[:], start=True, stop=True,
                    skip_group_check=True,
                )
                rb_sb = nsb.tile([D, 512], BF, tag="rb")
                nc.scalar.copy(rb_sb[:], rb_ps[:])
                ah = nsb.tile([D, 512], BF, tag="ah")
                nc.vector.tensor_mul(ah[:], pv_ps[:D, :], rb_sb[:])
                hi8 = nsb.tile([D, 512], F8, tag="hi8")
                nc.vector.tensor_copy(hi8[:], ah[:])
                lo8 = nsb.tile([D, 512], F8, tag="lo8")
                nc.vector.tensor_tensor(lo8[:], ah[:], hi8[:], op=OP.subtract)
                # repack hi/lo for the 128-row o-proj contraction
                for pi, (t, pd0, ps0, ln) in enumerate(qpieces[hh]):
                    eng = nc.sync if (hh + pi) % 2 == 0 else nc.gpsimd
                    eng.dma_start(
                        attnp_sb[pd0 : pd0 + ln, t, ssl],
                        hi8[ps0 : ps0 + ln, :],
                    )
                    eng.dma_start(
                        attnp_sb[pd0 : pd0 + ln, 3 + t, ssl],
                        lo8[ps0 : ps0 + ln, :],
                    )

            # ---------------- main loop ----------------------------------
            # chunk 0's prep runs serially (nothing to hide it under); its
            # v tiles pad the gate chain.  x^T chunk 1 prefetch is queued
            # behind the initial loads so prep(1), consumed as chunk-0
            # attention filler, finds its input ready.
            carried = {}
            prologue_qkv()
            ga0, gb0, gc0, gd0 = gate_thunks(0)
            de0, ro10, ro20 = rope_thunks(0)
            vt0 = [v_tile(ti) for ti in range(4)]

            def prefetch1():
                # x^T chunk 1, queued behind chunk 0's de-interleave DMAs
                for g in range(4):
                    ks = slice(g * 6 * 128, (g + 1) * 6 * 128)
                    eng = nc.sync if g % 2 == 0 else nc.gpsimd
                    eng.dma_start(
                        xth_sb[:, 1, g * 6 : (g + 1) * 6, :],
                        xth_d[ks, 512:1024].rearrange("(k p) c -> p k c", p=128),
                    )
                    eng.dma_start(
                        xtl_sb[:, 1, g * 6 : (g + 1) * 6, :],
                        xtl_d[ks, 512:1024].rearrange("(k p) c -> p k c", p=128),
                    )

            for th in [ga0, gb0, vt0[0], gc0, vt0[1], gd0, de0, prefetch1,
                       ro10, vt0[2], vt0[3], ro20]:
                th()

            for j in range(NS):
                # prefetch x^T for chunk j+2 (its buffer was freed when
                # chunk j's QKV, emitted during chunk j-1, finished)
                if j + 2 < NS:
                    pf = j + 2
                    nsl = slice(pf * 512, (pf + 1) * 512)
                    for g in range(4):
                        ks = slice(g * 6 * 128, (g + 1) * 6 * 128)
                        eng = nc.sync if g % 2 == 0 else nc.gpsimd
                        eng.dma_start(
                            xth_sb[:, pf % 2, g * 6 : (g + 1) * 6, :],
                            xth_d[ks, nsl].rearrange("(k p) c -> p k c", p=128),
                        )
                        eng.dma_start(
                            xtl_sb[:, pf % 2, g * 6 : (g + 1) * 6, :],
                            xtl_d[ks, nsl].rearrange("(k p) c -> p k c", p=128),
                        )
                if j == 0:
                    # o-proj weights: first consumer is oproj(0) in chunk 1
                    for t in range(6):
                        nc.sync.dma_start(
                            owp_sb[:, t, :], owp_d[:, t * HIDDEN : (t + 1) * HIDDEN]
                        )

                fills = []
                if "h3norm" in carried:
                    fills.append(carried.pop("h3norm"))
                # o-proj si tiles are deferred with a global balance: the
                # later chunks have longer attention streams, so they carry
                # more of the earlier chunks' o-proj work
                si_of = {1: [0, 1], 2: [2, 3, 4, 5], 3: [6, 7, 8, 9, 10, 11]}
                op = []
                for si in si_of.get(j, []):
                    op += oproj_thunks(si)
                pp = prep_thunks(j + 1) if j + 1 < NS else []
                # prep first: its gate/rope chains then only compete with
                # the early exps, and the chunk tail is pure o-proj fill;
                # hold back a few o-proj fills to cover the final norm
                # chain of the last chunk
                hold = op[-9:] if j == NS - 1 else []
                op = op[: len(op) - len(hold)]
                fills += pp + op

                attention(j, fills, carried)
                for th in hold:
                    th()

            for si in range(4 * (NS - 1), 4 * NS):
                for th in oproj_thunks(si):
                    th()

            if debug:
                for nm, t in [
                    ("dq", q_sb),
                    ("dk", k_sb),
                    ("dv", v_sb),
                    ("dqp", qp_sb),
                    ("dkm", km_sb),
                    ("dkx", kx_sb),
                    ("dattnp", attnp_sb),
                ]:
                    dd = nc.dram_tensor(
                        nm, list(t[:].shape), t[:].dtype, kind="ExternalOutput"
                    )
                    nc.sync.dma_start(dd[:], t[:])
    return nc


def _split8(x):
    f8 = ml_dtypes.float8_e4m3
    hi = x.astype(f8)
    lo = (x - hi.astype(np.float32)).astype(f8)
    return hi, lo


def _host_prep(hidden_states, cos, sin, qkv_w, o_w, gate_wq, gate_wk):
    bf = ml_dtypes.bfloat16
    X = np.asarray(hidden_states, np.float32).reshape(S, HIDDEN)
    qkv_w = np.asarray(qkv_w, np.float32)
    o_w = np.asarray(o_w, np.float32)
    cos = np.asarray(cos, np.float32)
    sin = np.asarray(sin, np.float32)

    # activations in hi+lo fp8 (residual split of the fp32 values);
    # weights are scaled x64 into fp8 range, divided back out on device
    xth, xtl = _split8(np.ascontiguousarray(X.T))
    scale = D**-0.5
    sgn = np.ones((D, 1), np.float32)
    sgn[: D // 2] = -1.0
    cosT = np.ascontiguousarray(cos.T)
    sinT = np.ascontiguousarray(sin.T)
    cosq = (cosT * scale / 64.0).astype(bf)
    sinq = (sinT * scale * sgn / 64.0).astype(bf)
    cosk = (cosT / 64.0).astype(bf)
    sink = (sinT * sgn / 64.0).astype(bf)

    # one-hot k-block indicator rows appended under k^T for mask-in-matmul
    kext = (np.arange(S)[None, :] // BLK == np.arange(NB)[:, None]).astype(
        np.float32
    )

    bcm_full = np.where(
        np.arange(NB)[None, :] <= np.arange(NB)[:, None], 0.0, -60.0
    ).astype(np.float32)
    bcm = np.ascontiguousarray(
        bcm_full.reshape(NS, 8, NB).transpose(1, 0, 2).reshape(8, NS * NB)
    )
    eye_full = np.eye(NB, dtype=np.float32)
    eyer = np.ascontiguousarray(
        eye_full.reshape(NS, 8, NB).transpose(1, 0, 2).reshape(8, NS * NB)
    )
    # cmask[p, col] = 1 if col - p >= 0 (within-tile causal for the
    # 128-col diagonal sub-block)
    p_i = np.arange(128)[:, None]
    col = np.arange(512)[None, :]
    cmask = (col - p_i >= 0).astype(np.float32).astype(bf)

    # k block mean is computed on-device as a SUM; fold 1/BLK into the
    # mean-pool half of gate_wk
    gwk_s = np.asarray(gate_wk, np.float32).copy()
    gwk_s[:D, :] *= 1.0 / BLK

    # gate_wq replicated per packed q row: gwqp[p, t*GH+g] = gate_wq[(128t+p)%D, g]
    gwq = np.asarray(gate_wq, np.float32)
    gwqp = np.zeros((128, 3 * GH), np.float32)
    for t in range(3):
        for p in range(128):
            r = 128 * t + p
            gwqp[p, t * GH : (t + 1) * GH] = gwq[r % D, :]

    common = dict(
        xth=xth,
        xtl=xtl,
        cosq=cosq,
        sinq=sinq,
        cosk=cosk,
        sink=sink,
        gwqp=gwqp,
        gwk=gwk_s,
        eye8=np.eye(8, dtype=np.float32),
        eyer=eyer,
        kext=kext.astype(bf),
        bcm=bcm,
        cmask=cmask,
    )
    maps = []
    for c in range(NCORES):
        wq = qkv_w[:, c * G * D : (c + 1) * G * D]
        wk = qkv_w[:, H * D + c * D : H * D + (c + 1) * D]
        wv = qkv_w[:, H * D + HK * D + c * D : H * D + HK * D + (c + 1) * D]
        ow = o_w[c * G * D : (c + 1) * G * D, :]  # [384, 3072]
        owt = ow.reshape(3, 128, HIDDEN).transpose(1, 0, 2) * 64.0
        owh, owl = _split8(owt)
        owp = np.concatenate(
            [owh.astype(np.float32), owl.astype(np.float32)], axis=1
        ).reshape(128, 6 * HIDDEN)
        wqkh, wqkl = _split8(np.concatenate([wq, wk], axis=1) * 64.0)
        wvh, wvl = _split8(wv * 64.0)
        maps.append(
            dict(
                common,
                wqkh=wqkh,
                wqkl=wqkl,
                wvh=wvh,
                wvl=wvl,
                owp=owp.astype(ml_dtypes.float8_e4m3),
            )
        )
    return maps


def _gather(results):
    acc = np.zeros((S, HIDDEN), np.float32)
    for r in results:
        acc += np.asarray(r["out_p"]).astype(np.float32)
    return acc.reshape(1, S, HIDDEN)


def _run(inputs, trace=False):
    global _prog
    if _prog is None:
        _prog = _build()
        if not _prog.is_finalized():
            _prog.finalize()
    from concourse import bass_utils

    maps = _host_prep(**inputs)
    res = bass_utils.run_bass_kernel_spmd(
        _prog, maps, list(range(NCORES)), trace=trace
    )
    return _gather(res.results), res


def kernel(**inputs):
    out, _ = _run(inputs, trace=False)
    return out
